# revision 1
# baseline (speedup 1.0000x reference)
"""CTC loss (warp-ctc semantics, size_average=True) on 8 Trainium2 NeuronCores.

Strategy (data-parallel over batch, 4 samples per core):
- Z[t,b] = sum_v exp(acts[t,b,v]) streamed as [128,8000] tiles; exp + free-dim
  sum fused in one ScalarE activation (accum_out). Host does log Z in float64.
- The alpha recursion runs in the LINEAR domain on unnormalized p~ = exp(acts
  at extended labels). States split into blank block (101) / label block (100),
  laid out [state-partition, (t,b)-free]. Using blank' = blank + shift(label)
  and label' = label + blank', each step is ONE TensorE matmul with a fixed
  shift stationary (no weight reloads; targets have no adjacent repeats) plus
  VectorE adds and the emission multiply; the [blank | label+blank] pre-add
  runs concurrently with the matmul so only add+mul sit on the serial chain.
  Every R steps the state-sum (ones-column matmuls) rescales alpha; factors
  are folded back in log-space on the host.
- The emission table (101 x T*8: gathered label acts + broadcast blank col)
  is host-prepared index prep; one DMA + one exp on device.
- Final: ll_b = log(alpha_fin) + sum log u - sum log Z  (host, float64);
  loss = -mean(ll).
"""

import sys
import types

import numpy as np

# ---- shim: provide antenv.axon_hooks (missing in this image) ----------------
_HOOK = [None]
try:
    import antenv.axon_hooks  # noqa: F401
except ImportError:
    try:
        from trn_agent_boot.trn_boot import _ntff_profile_via_ctypes

        _HOOK[0] = _ntff_profile_via_ctypes("/opt/axon/libaxon_pjrt.so")
    except Exception:
        pass
    _m = types.ModuleType("antenv.axon_hooks")
    _m.get_axon_ntff_profile_hook = lambda: _HOOK[0]
    _m.set_axon_ntff_profile_hook = lambda h: _HOOK.__setitem__(0, h)
    sys.modules["antenv.axon_hooks"] = _m
# -----------------------------------------------------------------------------

import concourse.bass as bass
import concourse.mybir as mybir
import concourse.tile as tile
from concourse.bass_utils import run_bass_kernel_spmd
from concourse.vector_clock import ScopedClock


# ---- walrus-compat patches: this walrus rejects Drains with >1 sem wait -----
def _my_drain_and_barrier(self, tick_clock, wait_clock):
    nc = self.nc
    dummy = nc.sync.nop(nofuse=True)
    wait_clock.add_sem_waits(dummy.ins, ScopedClock({None: tick_clock.global_clock}))
    si = dummy.ins.sync_info
    waits = list(si.on_wait) if si is not None else []
    if si is not None and len(waits) > 1:
        dummy.ins.sync_info = mybir.SyncInfo(
            on_wait=[waits[0]], on_update=list(si.on_update)
        )
        for w in waits[1:]:
            n = nc.sync.nop(nofuse=True)
            n.ins.sync_info = mybir.SyncInfo(on_wait=[w], on_update=[])
    nc.sync.drain()
    nc.all_engine_barrier()
    assert self.sems is not None
    popped = nc._tile_sem_poison_stack.pop()
    assert popped is self._sem_poison
    nc.clear_and_free_semaphores(list(self.sems.allocated().values()))
    nc.all_engine_barrier()


def _my_multi_engine_barrier(self, engines):
    # bare per-engine drains (this walrus rejects waits on Drain) followed by
    # an EVSEM sem-only all-engine barrier for the cross-engine sync.
    for e in engines:
        self.engines[e].drain()
    for inst in self._sem_only_all_engine_barrier_insts(f"aeb{self.next_id()}"):
        self.engines[inst.engine].add_instruction(inst)


tile.TileContext._drain_and_barrier = _my_drain_and_barrier
bass.Bass.multi_engine_barrier = _my_multi_engine_barrier


def _split_multiwait(nc):
    """This walrus build encodes at most one sync-wait per instruction; hoist
    extra waits onto preceding nofuse NOPs on the same engine."""
    n_new = 0
    for fn in nc.m.functions:
        for blk in fn.blocks:
            insts = blk.instructions
            i = 0
            while i < len(insts):
                ins = insts[i]
                si = getattr(ins, "sync_info", None)
                if si is not None and si.on_wait and len(si.on_wait) > 1:
                    waits = list(si.on_wait)
                    ins.sync_info = mybir.SyncInfo(
                        on_wait=[waits[-1]], on_update=list(si.on_update)
                    )
                    new_nops = []
                    for w in waits[:-1]:
                        nop = mybir.InstNoOp(
                            name=f"{ins.name}_wsplit{n_new}",
                            engine=ins.engine,
                            sync_info=mybir.SyncInfo(on_wait=[w], on_update=[]),
                            bass_nofuse=True,
                        )
                        n_new += 1
                        new_nops.append(nop)
                    insts[i:i] = new_nops
                    i += len(new_nops)
                i += 1
    return nc
# -----------------------------------------------------------------------------

T, B, V, L = 512, 32, 8000, 100
NCORES = 8
NB = B // NCORES  # 4 samples per core
W = 2 * NB  # alpha free width: cols 0..NB-1 blank block, NB..2NB-1 label block
NBLK = L + 1  # blank states
NLAB = L  # label states
RSC = 16  # rescale every RSC steps
F32 = mybir.dt.float32
I32 = mybir.dt.int32


def n_rescales(t_steps):
    return len([t for t in range(1, t_steps) if t % RSC == 0 and t != t_steps - 1])


def build_weights():
    """Static 0/1 lhsT weight matrices [K, M] for the per-step matmuls.

    psum[:, 0:NB]   = w_b0.T @ blank + w_n0.T @ label   (new blank block)
    psum[:, NB:2NB] = w_b1.T @ blank + w_n1.T @ label   (new label block)
    blank'[j] = blank[j] + label[j-1]; label'[j] = label[j] + blank[j] + label[j-1]
    """
    w_b0 = np.zeros((NBLK, NBLK), np.float32)
    w_n0 = np.zeros((NLAB, NBLK), np.float32)
    w_b1 = np.zeros((NBLK, NBLK), np.float32)
    w_n1 = np.zeros((NLAB, NBLK), np.float32)
    for k in range(NBLK):
        w_b0[k, k] = 1.0
        if k < NLAB:
            w_b1[k, k] = 1.0
    for k in range(NLAB):
        w_n0[k, k + 1] = 1.0
        w_n1[k, k] = 1.0
        if k + 1 < NLAB:
            w_n1[k, k + 1] = 1.0
    return w_b0, w_n0, w_b1, w_n1


def build_program(t_steps=T, split=True, do_stream=True, do_rec=True):
    """Build the per-core Bass program (identical for all cores)."""
    nc = bass.Bass("TRN2", target_bir_lowering=False, debug=False)
    ntile = NB * (t_steps // 128)
    nresc = n_rescales(t_steps)

    acts_d = nc.dram_tensor("acts", [NB * t_steps, V], F32, kind="ExternalInput")
    pg_d = nc.dram_tensor("pg", [NBLK, t_steps * W], F32, kind="ExternalInput")
    w_n0_d = nc.dram_tensor("w_n0", [NLAB, NBLK], F32, kind="ExternalInput")
    e0mask_d = nc.dram_tensor("e0mask", [NBLK, W], F32, kind="ExternalInput")

    zout_d = nc.dram_tensor("zout", [ntile, 128], F32, kind="ExternalOutput")
    afin_d = nc.dram_tensor("afin", [NBLK, W], F32, kind="ExternalOutput")
    ubuf_d = nc.dram_tensor("ubuf", [1, (nresc + 1) * W], F32, kind="ExternalOutput")

    with tile.TileContext(nc) as tc:
        with (
            tc.tile_pool(name="stream", bufs=2) as stream_pool,
            tc.tile_pool(name="escratch", bufs=1) as escratch_pool,
            tc.tile_pool(name="zpool", bufs=2) as zpool,
            tc.tile_pool(name="singles", bufs=1) as singles,
            tc.tile_pool(name="alpha", bufs=6) as alpha_pool,
            tc.tile_pool(name="mainpsum", bufs=4, space="PSUM") as mainpsum,
            tc.tile_pool(name="bpsum", bufs=2, space="PSUM") as bpsum,
            tc.tile_pool(name="upsum", bufs=2, space="PSUM") as upsum,
        ):
            # ---- static small inputs -> SBUF --------------------------------
            w_n0 = singles.tile([NLAB, NBLK], F32)
            e0mask = singles.tile([NBLK, W], F32)
            ones_row = singles.tile([1, NBLK], F32)  # lhsT for bcast [1]x[101]
            ones_colk = singles.tile([NBLK, 1], F32)  # lhsT for sums [101]x[1]
            nc.sync.dma_start(out=w_n0, in_=w_n0_d[:, :])
            nc.sync.dma_start(out=e0mask, in_=e0mask_d[:, :])
            nc.vector.memset(ones_row, 1.0)
            nc.vector.memset(ones_colk, 1.0)

            # ---- emission table p~ [state 0..100, (t, col)] -----------------
            # host supplies pg = raw acts at extended labels (blank cols are
            # the blank activation broadcast across state partitions).
            phat_raw = singles.tile([NBLK, t_steps * W], F32)
            phat = singles.tile([NBLK, t_steps * W], F32)
            nc.sync.dma_start(out=phat_raw, in_=pg_d[:, :])
            nc.scalar.activation(phat, phat_raw, mybir.ActivationFunctionType.Exp)

            # ---- streaming Z = sum_v exp(acts) ------------------------------
            for it in range(ntile if do_stream else 0):
                tile_a = stream_pool.tile([128, V], F32, tag="acts")
                nc.sync.dma_start(out=tile_a, in_=acts_d[it * 128 : (it + 1) * 128, :])
                e_t = escratch_pool.tile([128, V], F32, tag="escr")
                z_t = zpool.tile([128, 1], F32, tag="z")
                nc.scalar.activation(
                    e_t, tile_a, mybir.ActivationFunctionType.Exp, accum_out=z_t
                )
                nc.sync.dma_start(out=zout_d[it : it + 1, :], in_=z_t)

            # ---- alpha recursion -------------------------------------------
            ubuf = singles.tile([1, (nresc + 1) * W], F32)
            nc.vector.memset(ubuf, 1.0)

            alpha = alpha_pool.tile([NBLK, W], F32, tag="alpha")
            nc.vector.tensor_mul(alpha, phat[:, 0:W], e0mask)

            n_resc = 0
            for t in range(1, t_steps if do_rec else 1):
                # ps = shift(label) in both col blocks; stationary w_n0 is the
                # only per-step weight -> stays resident on the PE.
                ps = mainpsum.tile([NBLK, W], F32, tag="mps")
                lab_dup = bass.AP(
                    tensor=alpha.tensor,
                    offset=alpha[0:NLAB, NB:W].offset,
                    ap=[list(alpha[0:NLAB, NB:W].ap[0]), [0, 2], [1, NB]],
                )
                nc.tensor.matmul(ps, w_n0, lab_dup, start=True, stop=True)
                # yprep = [blank | label+blank] runs concurrently with the
                # matmul; the serial tail after PE is just add + mult.
                yprep = alpha_pool.tile([NBLK, W], F32, tag="yprep")
                nc.vector.tensor_copy(yprep[:, 0:NB], alpha[:, 0:NB])
                nc.vector.tensor_add(
                    yprep[:, NB:W], alpha[:, NB:W], alpha[:, 0:NB]
                )
                y = alpha_pool.tile([NBLK, W], F32, tag="yprep")
                nc.vector.tensor_add(y, yprep, ps[0:NBLK, :])
                alpha_next = alpha_pool.tile([NBLK, W], F32, tag="alpha")
                nc.vector.tensor_mul(
                    alpha_next, y, phat[:, t * W : (t + 1) * W]
                )
                alpha = alpha_next

                if t % RSC == 0 and t != t_steps - 1:
                    # u = sum_s alpha at partition 0 via ones-column matmuls
                    pu = upsum.tile([1, W], F32, tag="ups")
                    nc.tensor.matmul(
                        pu[:, 0:NB], ones_colk, alpha[0:NBLK, 0:NB], start=True, stop=False
                    )
                    nc.tensor.matmul(
                        pu[:, 0:NB],
                        ones_colk[0:NLAB, :],
                        alpha[0:NLAB, NB:W],
                        start=False,
                        stop=True,
                    )
                    nc.vector.tensor_copy(
                        ubuf[0:1, n_resc * W : n_resc * W + NB], pu[0:1, 0:NB]
                    )
                    rrec = singles.tile([1, NB], F32, tag="rrec")
                    nc.vector.reciprocal(rrec, pu[0:1, 0:NB])
                    pb = bpsum.tile([NBLK, W], F32, tag="rbc")
                    nc.tensor.matmul(pb[:, 0:NB], ones_row, rrec, start=True, stop=True)
                    nc.tensor.matmul(pb[:, NB:W], ones_row, rrec, start=True, stop=True)
                    alpha_r = alpha_pool.tile([NBLK, W], F32, tag="alpha")
                    nc.vector.tensor_mul(alpha_r, alpha, pb)
                    alpha = alpha_r
                    n_resc += 1

            nc.sync.dma_start(out=afin_d[:, :], in_=alpha)
            nc.sync.dma_start(out=ubuf_d[:, :], in_=ubuf)
    if split:
        _split_multiwait(nc)
    return nc


_NC_CACHE = {}


def _get_program(t_steps=T):
    if t_steps not in _NC_CACHE:
        _NC_CACHE[t_steps] = build_program(t_steps)
    return _NC_CACHE[t_steps]


def make_in_maps(acts, targets, t_steps=T):
    _, w_n0, _, _ = build_weights()
    e0mask = np.zeros((NBLK, W), np.float32)
    e0mask[0, :] = 1.0
    in_maps = []
    for c in range(NCORES):
        bs = slice(c * NB, (c + 1) * NB)
        acts_c = np.ascontiguousarray(
            acts[:t_steps, bs, :].transpose(1, 0, 2).reshape(NB * t_steps, V)
        )
        tg = targets[bs]  # [NB, L]
        a = acts[:t_steps, bs, :]  # [T, NB, V]
        pg = np.zeros((NBLK, t_steps, W), np.float32)
        # label cols: pg[l, t, NB+b] = a[t, b, tg[b, l]]
        gat = a[:, np.arange(NB)[:, None], tg]  # [NB, L] adv-idx -> [T, NB, L]
        pg[0:NLAB, :, NB : NB + NB] = gat.transpose(2, 0, 1)
        # blank cols: pg[:, t, b] = a[t, b, 0] broadcast over states
        pg[:, :, 0:NB] = a[:, :, 0][None, :, :]
        pg[NLAB:, :, NB:W] = -30.0
        pg = np.ascontiguousarray(pg.reshape(NBLK, t_steps * W))
        in_maps.append(
            {
                "acts": acts_c,
                "pg": pg,
                "w_n0": w_n0,
                "e0mask": e0mask,
            }
        )
    return in_maps


def finalize(results, t_steps=T):
    """Host-side combine: per-sample log-likelihoods -> scalar loss (f64)."""
    nresc = n_rescales(t_steps)
    ntchunk = t_steps // 128
    lls = []
    for c in range(NCORES):
        out = results[c]
        zout = out["zout"].astype(np.float64)  # [ntile, 128]
        afin = out["afin"].astype(np.float64)  # [NBLK, W]
        ubuf = out["ubuf"].astype(np.float64).reshape(-1, W)  # [nresc+1, W]
        for b in range(NB):
            logz = np.log(zout[b * ntchunk : (b + 1) * ntchunk, :]).sum()
            logu = np.log(ubuf[:nresc, b]).sum() if nresc else 0.0
            fin = afin[NBLK - 1, b] + afin[NLAB - 1, NB + b]
            lls.append(np.log(fin) + logu - logz)
    return -np.sum(lls) / B


def kernel(acts, targets, act_lens, label_lens):
    acts = np.asarray(acts, np.float32)
    targets = np.asarray(targets).astype(np.int64)
    act_lens = np.asarray(act_lens)
    label_lens = np.asarray(label_lens)
    assert acts.shape == (T, B, V), acts.shape
    assert targets.shape == (B, L)
    assert (act_lens == T).all() and (label_lens == L).all(), "only full lens supported"
    assert (targets[:, 1:] != targets[:, :-1]).all(), "adjacent repeats unsupported"

    nc = _get_program(T)
    in_maps = make_in_maps(acts, targets, T)
    res = run_bass_kernel_spmd(nc, in_maps, core_ids=list(range(NCORES)))
    return np.float32(finalize(res.results, T))


if __name__ == "__main__":
    rng = np.random.default_rng(0)
    acts = rng.standard_normal((T, B, V)).astype(np.float32)
    targets = rng.integers(1, V, (B, L)).astype(np.int32)
    for bb in range(B):
        while (targets[bb, 1:] == targets[bb, :-1]).any():
            targets[bb] = rng.integers(1, V, (L,)).astype(np.int32)
    act_lens = np.full(B, T, np.int32)
    label_lens = np.full(B, L, np.int32)
    out = kernel(acts, targets, act_lens, label_lens)
    print("kernel loss:", out)
    from ctc_numpy import ctc_ref_numpy

    ref = ctc_ref_numpy(acts, targets, act_lens, label_lens)
    print("ref    loss:", ref, " rel err:", abs(out - ref) / abs(ref))



# revision 6
# speedup vs baseline: 2.6260x; 2.6260x over previous
"""CTC loss (warp-ctc semantics, size_average=True) on 8 Trainium2 NeuronCores.

Strategy (data-parallel over batch, 4 samples per core):
- Z[t,b] = sum_v exp(acts[t,b,v]) streamed as [128,8000] f32 tiles over TWO
  hardware DMA queues (sync + scalar); exp + free-dim sum fused in one ScalarE
  activation (accum_out, f32). Host does log Z in float64.
- The alpha recursion runs in the LINEAR domain, bf16, with STATES ON THE FREE
  AXIS (partition shifts are illegal on compute engines; free-dim offsets are
  free). 8 partition rows = 4 samples x {fw, bw}; free = [guard, blank 0..100,
  guard, guard, label 0..99] (width 204). Zero guards make the state shift a
  plain free-dim offset.
- Forward and backward DPs run SIMULTANEOUSLY in the same instructions
  (backward CTC == forward CTC on time-reversed, state-flipped data), meeting
  in the middle: 255 fused steps instead of 511. Per step exactly 4 DVE ops:
    ADD1: y_blank = blank + shift(label)
    ADD2: y_label = y_blank + label
    MULb: A_blank = y_blank * pb_k      (tensor_scalar, per-row blank emission)
    MULl: A_label = y_label * pl_k
  No TensorE, no cross-engine syncs on the serial chain.
- Rescale every RSC steps: the two MULs' accum_out give state sums free;
  reciprocal applied via tensor_scalar. Factors folded back in log on host.
- Emission tables (pre-exp'd on host: pgL bf16 [8,256*100], pgB f32 [8,256])
  and the init state a0 are host-prepared; three small DMAs.
- Final: ll_b = log(sum_s y_fw[s]*g_bw[s]) + sum log u - sum log Z (host f64);
  loss = -mean(ll).
"""

import sys
import types

import numpy as np
import ml_dtypes

# ---- shim: provide antenv.axon_hooks (missing in this image) ----------------
_HOOK = [None]
try:
    import antenv.axon_hooks  # noqa: F401
except ImportError:
    try:
        from trn_agent_boot.trn_boot import _ntff_profile_via_ctypes

        _HOOK[0] = _ntff_profile_via_ctypes("/opt/axon/libaxon_pjrt.so")
    except Exception:
        pass
    _m = types.ModuleType("antenv.axon_hooks")
    _m.get_axon_ntff_profile_hook = lambda: _HOOK[0]
    _m.set_axon_ntff_profile_hook = lambda h: _HOOK.__setitem__(0, h)
    sys.modules["antenv.axon_hooks"] = _m
# -----------------------------------------------------------------------------

import concourse.bass as bass
import concourse.mybir as mybir
import concourse.tile as tile
from concourse.bass_utils import run_bass_kernel_spmd
from concourse.vector_clock import ScopedClock


# ---- walrus-compat patches: this walrus rejects Drains with >1 sem wait -----
def _my_drain_and_barrier(self, tick_clock, wait_clock):
    nc = self.nc
    dummy = nc.sync.nop(nofuse=True)
    wait_clock.add_sem_waits(dummy.ins, ScopedClock({None: tick_clock.global_clock}))
    si = dummy.ins.sync_info
    waits = list(si.on_wait) if si is not None else []
    if si is not None and len(waits) > 1:
        dummy.ins.sync_info = mybir.SyncInfo(
            on_wait=[waits[0]], on_update=list(si.on_update)
        )
        for w in waits[1:]:
            n = nc.sync.nop(nofuse=True)
            n.ins.sync_info = mybir.SyncInfo(on_wait=[w], on_update=[])
    nc.sync.drain()
    nc.all_engine_barrier()
    assert self.sems is not None
    popped = nc._tile_sem_poison_stack.pop()
    assert popped is self._sem_poison
    nc.clear_and_free_semaphores(list(self.sems.allocated().values()))
    nc.all_engine_barrier()


def _my_multi_engine_barrier(self, engines):
    for e in engines:
        self.engines[e].drain()
    for inst in self._sem_only_all_engine_barrier_insts(f"aeb{self.next_id()}"):
        self.engines[inst.engine].add_instruction(inst)


tile.TileContext._drain_and_barrier = _my_drain_and_barrier
bass.Bass.multi_engine_barrier = _my_multi_engine_barrier


def _split_multiwait(nc):
    """This walrus build encodes at most one sync-wait per instruction; hoist
    extra waits onto preceding nofuse NOPs on the same engine."""
    n_new = 0
    for fn in nc.m.functions:
        for blk in fn.blocks:
            insts = blk.instructions
            i = 0
            while i < len(insts):
                ins = insts[i]
                si = getattr(ins, "sync_info", None)
                if si is not None and si.on_wait and len(si.on_wait) > 1:
                    waits = list(si.on_wait)
                    ins.sync_info = mybir.SyncInfo(
                        on_wait=[waits[-1]], on_update=list(si.on_update)
                    )
                    new_nops = []
                    for w in waits[:-1]:
                        nop = mybir.InstNoOp(
                            name=f"{ins.name}_wsplit{n_new}",
                            engine=ins.engine,
                            sync_info=mybir.SyncInfo(on_wait=[w], on_update=[]),
                            bass_nofuse=True,
                        )
                        n_new += 1
                        new_nops.append(nop)
                    insts[i:i] = new_nops
                    i += len(new_nops)
                i += 1
    return nc
# -----------------------------------------------------------------------------

T, B, V, L = 512, 32, 8000, 100
NCORES = 8
NB = B // NCORES  # 4 samples per core
KS = 255  # fused fw+bw steps (t_fw = 1..255, t_bw = 510..256)
NK = 256  # table slots (slot 0 unused; init comes via a0 DMA)
FW = 204  # free width: f0 guard, f1..101 blank j=0..100, f102/103 guard, f104..203 label j=0..99
FB0, FL0 = 1, 104
RSC = 32  # rescale every RSC steps
NSITES = len(range(RSC, KS, RSC))  # 7
NTILE = NB * (T // 128)  # 16 streaming tiles per core
F32 = mybir.dt.float32
BF16 = mybir.dt.bfloat16
BFNP = ml_dtypes.bfloat16
MULT = mybir.AluOpType.mult


def build_program(t_steps=T):
    """Build the per-core Bass program (identical for all cores)."""
    assert t_steps == T
    nc = bass.Bass("TRN2", target_bir_lowering=False, debug=False)

    acts_d = nc.dram_tensor("acts", [NB * T, V], F32, kind="ExternalInput")
    pgl_d = nc.dram_tensor("pgl", [8, NK * L], BF16, kind="ExternalInput")
    pgb_d = nc.dram_tensor("pgb", [8, NK], F32, kind="ExternalInput")
    a0_d = nc.dram_tensor("a0", [8, FW], BF16, kind="ExternalInput")

    zacc_d = nc.dram_tensor("zacc", [128, NTILE], F32, kind="ExternalOutput")
    afin_d = nc.dram_tensor("afin", [8, FW], BF16, kind="ExternalOutput")
    yfin_d = nc.dram_tensor("yfin", [8, FW], BF16, kind="ExternalOutput")
    ubuf_d = nc.dram_tensor("ubuf", [8, 2 * NSITES], F32, kind="ExternalOutput")

    with tile.TileContext(nc) as tc:
        with (
            tc.tile_pool(name="singles", bufs=1) as singles,
            tc.tile_pool(name="stream", bufs=3) as stream_pool,
            tc.tile_pool(name="escratch", bufs=1) as escratch_pool,
        ):
            # ---- emission tables + init state -> SBUF -----------------------
            pgl = singles.tile([8, NK * L], BF16)
            pgb = singles.tile([8, NK], F32)
            A = singles.tile([8, FW], BF16)
            Y = singles.tile([8, FW], BF16)
            rt = singles.tile([8, 2], F32)
            ub = singles.tile([8, 2 * NSITES], F32)
            nc.sync.dma_start(out=pgl, in_=pgl_d[:, :])
            nc.sync.dma_start(out=pgb, in_=pgb_d[:, :])
            nc.vector.memset(Y, 1.0)  # guards f0/f102/f103 never read in Y
            nc.vector.memset(rt, 1.0)
            nc.sync.dma_start(out=A[:, :], in_=a0_d[:, :])

            site = 0
            for k in range(1, KS + 1):
                # ADD1: y_blank[j] = blank[j] + label[j-1], j = 0..100
                nc.vector.tensor_add(
                    Y[:, FB0 : FB0 + 101], A[:, FB0 : FB0 + 101], A[:, FL0 - 1 : FL0 + 100]
                )
                # ADD2: y_label[j] = y_blank[j] + label[j], j = 0..99
                nc.vector.tensor_add(
                    Y[:, FL0 : FL0 + 100], Y[:, FB0 : FB0 + 100], A[:, FL0 : FL0 + 100]
                )
                if k % RSC == 0 and k < KS:
                    nc.vector.tensor_scalar(
                        A[:, FB0 : FB0 + 101], Y[:, FB0 : FB0 + 101],
                        pgb[:, k : k + 1], 0.0, op0=MULT,
                        op1=mybir.AluOpType.add,
                        accum_out=ub[:, 2 * site : 2 * site + 1],
                    )
                    nc.vector.scalar_tensor_tensor(
                        A[:, FL0 : FL0 + 100], Y[:, FL0 : FL0 + 100], 0.0,
                        pgl[:, k * L : k * L + 100],
                        op0=mybir.AluOpType.add, op1=MULT,
                        accum_out=ub[:, 2 * site + 1 : 2 * site + 2],
                    )
                    nc.vector.tensor_add(
                        rt[:, 1:2],
                        ub[:, 2 * site : 2 * site + 1],
                        ub[:, 2 * site + 1 : 2 * site + 2],
                    )
                    nc.vector.reciprocal(rt[:, 0:1], rt[:, 1:2])
                    nc.vector.tensor_scalar_mul(A[:, :], A[:, :], rt[:, 0:1])
                    site += 1
                else:
                    nc.vector.tensor_scalar(
                        A[:, FB0 : FB0 + 101], Y[:, FB0 : FB0 + 101],
                        pgb[:, k : k + 1], None, op0=MULT,
                    )
                    nc.vector.tensor_mul(
                        A[:, FL0 : FL0 + 100], Y[:, FL0 : FL0 + 100],
                        pgl[:, k * L : k * L + 100],
                    )
            assert site == NSITES

            # final half-step: y_{KS+1} = ADD(A_KS) for the host dot product
            nc.vector.tensor_add(
                Y[:, FB0 : FB0 + 101], A[:, FB0 : FB0 + 101], A[:, FL0 - 1 : FL0 + 100]
            )
            nc.vector.tensor_add(
                Y[:, FL0 : FL0 + 100], Y[:, FB0 : FB0 + 100], A[:, FL0 : FL0 + 100]
            )
            nc.sync.dma_start(out=afin_d[:, :], in_=A)
            nc.sync.dma_start(out=yfin_d[:, :], in_=Y)
            nc.sync.dma_start(out=ubuf_d[:, :], in_=ub)

            # ---- streaming Z = sum_v exp(acts), two HW DMA queues -----------
            ztile = singles.tile([128, NTILE], F32)
            tiles = {}
            for it in range(min(3, NTILE)):
                ta = stream_pool.tile([128, V], F32, tag="acts")
                eng = nc.sync if it % 2 == 0 else nc.scalar
                eng.dma_start(out=ta, in_=acts_d[it * 128 : (it + 1) * 128, :])
                tiles[it] = ta
            for it in range(NTILE):
                ta = tiles.pop(it)
                e_t = escratch_pool.tile([128, V], BF16, tag="escr")
                nc.scalar.activation(
                    e_t, ta, mybir.ActivationFunctionType.Exp,
                    accum_out=ztile[:, it : it + 1],
                )
                nxt = it + 3
                if nxt < NTILE:
                    tb = stream_pool.tile([128, V], F32, tag="acts")
                    eng = nc.sync if nxt % 2 == 0 else nc.scalar
                    eng.dma_start(out=tb, in_=acts_d[nxt * 128 : (nxt + 1) * 128, :])
                    tiles[nxt] = tb
            nc.sync.dma_start(out=zacc_d[:, :], in_=ztile)
    _split_multiwait(nc)
    return nc


_NC_CACHE = {}


def _get_program(t_steps=T):
    if t_steps not in _NC_CACHE:
        _NC_CACHE[t_steps] = build_program(t_steps)
    return _NC_CACHE[t_steps]


def make_in_maps(acts, targets, t_steps=T):
    assert t_steps == T
    in_maps = []
    karr = np.arange(NK)
    bidx = np.arange(NB)
    for c in range(NCORES):
        bs = slice(c * NB, (c + 1) * NB)
        acts_c = np.ascontiguousarray(
            acts[:, bs, :].transpose(1, 0, 2).reshape(NB * T, V)
        )
        tg = np.asarray(targets[bs], np.int64)  # [NB, L]
        a = acts[:, bs, :]  # [T, NB, V] f32

        # blank emissions: [8, NK] f32 (rows 0:4 fw t=k, rows 4:8 bw t=511-k)
        pgb = np.empty((8, NK), np.float32)
        pgb[0:4, :] = np.exp(a[karr, :, 0]).T
        pgb[4:8, :] = np.exp(a[T - 1 - karr, :, 0]).T
        # label emissions: [8, NK, L]
        lab_fw = a[karr[:, None, None], bidx[None, :, None], tg[None, :, :]]
        lab_bw = a[
            (T - 1 - karr)[:, None, None], bidx[None, :, None], tg[None, :, ::-1]
        ]
        pgl = np.empty((8, NK, L), np.float32)
        pgl[0:4] = np.exp(lab_fw).transpose(1, 0, 2)
        pgl[4:8] = np.exp(lab_bw).transpose(1, 0, 2)
        # init state a0 [8, FW]
        a0 = np.zeros((8, FW), np.float32)
        for b in range(NB):
            a0[b, FB0] = np.exp(a[0, b, 0])
            a0[b, FL0] = np.exp(a[0, b, tg[b, 0]])
            a0[4 + b, FB0] = np.exp(a[T - 1, b, 0])
            a0[4 + b, FL0] = np.exp(a[T - 1, b, tg[b, L - 1]])
        in_maps.append(
            {
                "acts": acts_c,
                "pgl": np.ascontiguousarray(pgl.reshape(8, NK * L).astype(BFNP)),
                "pgb": pgb,
                "a0": a0.astype(BFNP),
            }
        )
    return in_maps


def finalize(results, t_steps=T):
    """Host-side combine: per-sample log-likelihoods -> scalar loss (f64)."""
    assert t_steps == T
    ntchunk = T // 128
    j101 = np.arange(101)
    j100 = np.arange(100)
    lls = []
    for c in range(NCORES):
        out = results[c]
        zacc = np.asarray(out["zacc"], np.float64)  # [128, NTILE]
        A = np.asarray(out["afin"], np.float64)  # [8, FW]
        Yf = np.asarray(out["yfin"], np.float64)  # [8, FW]
        ub = np.asarray(out["ubuf"], np.float64).reshape(8, NSITES, 2)
        logs = np.log(ub.sum(axis=2)).sum(axis=1)  # [8]
        for b in range(NB):
            zb = np.concatenate(
                [zacc[:, b * ntchunk + i] for i in range(ntchunk)]
            )  # [T]
            logz = np.log(zb).sum()
            # fw y_256 (row b) dotted with state-flipped bw g_256 (row 4+b)
            dot = (Yf[b, FB0 + j101] * A[4 + b, FB0 + 100 - j101]).sum()
            dot += (Yf[b, FL0 + j100] * A[4 + b, FL0 + 99 - j100]).sum()
            ll = np.log(dot) + logs[b] + logs[4 + b] - logz
            lls.append(ll)
    return -np.sum(lls) / B


def kernel(acts, targets, act_lens, label_lens):
    acts = np.asarray(acts, np.float32)
    targets = np.asarray(targets).astype(np.int64)
    act_lens = np.asarray(act_lens)
    label_lens = np.asarray(label_lens)
    assert acts.shape == (T, B, V), acts.shape
    assert targets.shape == (B, L)
    assert (act_lens == T).all() and (label_lens == L).all(), "only full lens supported"
    assert (targets[:, 1:] != targets[:, :-1]).all(), "adjacent repeats unsupported"

    nc = _get_program(T)
    in_maps = make_in_maps(acts, targets, T)
    res = run_bass_kernel_spmd(nc, in_maps, core_ids=list(range(NCORES)))
    return np.float32(finalize(res.results, T))


if __name__ == "__main__":
    rng = np.random.default_rng(0)
    acts = rng.standard_normal((T, B, V)).astype(np.float32)
    targets = rng.integers(1, V, (B, L)).astype(np.int32)
    for bb in range(B):
        while (targets[bb, 1:] == targets[bb, :-1]).any():
            targets[bb] = rng.integers(1, V, (L,)).astype(np.int32)
    act_lens = np.full(B, T, np.int32)
    label_lens = np.full(B, L, np.int32)
    out = kernel(acts, targets, act_lens, label_lens)
    print("kernel loss:", out)
    from ctc_numpy import ctc_ref_numpy

    ref = ctc_ref_numpy(acts, targets, act_lens, label_lens)
    print("ref    loss:", ref, " rel err:", abs(out - ref) / abs(ref))
